# revision 16
# baseline (speedup 1.0000x reference)
"""HSMM generative forward — Bass kernel v2 for 8 TRN2 NeuronCores.

v2 reshards the cond MLP with collectives to kill the 8x-replicated W_c2
HBM streaming that bounded v1:

  stage0  pi = softmax(u @ W_init)            per-core own 128 examples
  stage1  hT = relu(W_c1^T u^T)               own examples -> DRAM
  AG      AllGather h                          -> h for all 1024 examples
  stage2  cond^T slice = W_c2[:, slice]^T @ h  ALL examples, 1/8 of cols
          (per core: 16 cond-col tiles == transition-row index k)
  A2A     AllToAll cond                        -> own 128 examples, all cols
  stage3  per example: trans = tsc + cf ct^T; P = fp8(exp(z - rowmax)),
          Z = rowsum(fp8 P) [quantize-then-normalize], invZ = 1/Z
  phase B recurrence in prob space with invZ folded into the u-ring
          (astar' = invZ * P_unorm^T a), fp8 stationary weights.

W_c2 HBM traffic per core: 34 MB (was 272 MB). P matvec weight loads are
fp8 (FWL streams 4 cols/cycle vs 2 for bf16).
"""

import numpy as np
import ml_dtypes

BF16 = ml_dtypes.bfloat16

K = 128
A_DIM = 64
L = 6
T = 60
BSZ = 1024
TH2 = 1024
B = 128           # batch shard per core
N_CORES = 8
NEGINF = -1e38
LEN_LP = float(-np.log(L))

I1_TILES = 9      # ceil((1024+1)/128): uniqenc + bias row
I2_TILES = 65     # ceil((8192+1)/128): h + bias row
NT = 16           # cond-col tiles per core (== local k-tiles)
RENORM_EVERY = 6

_CACHE = {}
_WSPLIT_UID = [0]


def _split_excess_waits(nc, max_waits=1):
    """Walrus in this container rejects >1 sync wait per instruction; hoist
    excess waits onto preceding same-engine NoOps."""
    from concourse import mybir

    n_split = 0
    for fn in nc.m.functions:
        for blk in fn.blocks:
            insts = blk.instructions
            new_insts = []
            changed = False
            for inst in insts:
                si = getattr(inst, "sync_info", None)
                waits = list(si.on_wait) if si is not None else []
                if len(waits) > max_waits:
                    changed = True
                    n_split += 1
                    extra = waits[:-max_waits]
                    keep = waits[len(extra):]
                    for i in range(0, len(extra), max_waits):
                        chunk = extra[i:i + max_waits]
                        _WSPLIT_UID[0] += 1
                        nop = mybir.InstNoOp(
                            name=f"I-wsplit-{_WSPLIT_UID[0]}", ins=[], outs=[])
                        nop.engine = inst.engine
                        nop.sync_info = mybir.SyncInfo(on_wait=chunk,
                                                       on_update=[])
                        new_insts.append(nop)
                    si.on_wait = keep
                new_insts.append(inst)
            if changed:
                blk.instructions = new_insts
    return n_split


def _stage0_pi(nc, tc, bass, mybir, AF, uT_sb, Wi_sb, smallp, ident_f32,
               u_ring, sfx=""):
    f32 = mybir.dt.float32
    with tc.tile_pool(name="ps0" + sfx, bufs=2,
                      space=bass.MemorySpace.PSUM) as ps0:
        ps_z = ps0.tile([128, 128], f32, tag="ps_z")
        for i in range(I1_TILES):
            nc.tensor.matmul(ps_z[:], uT_sb[:, i, :], Wi_sb[:, i, :],
                             start=(i == 0), stop=(i == I1_TILES - 1))
        mx = smallp.tile([128, 1], f32, tag="mx")
        nc.vector.reduce_max(out=mx[:], in_=ps_z[:], axis=mybir.AxisListType.X)
        negm = smallp.tile([128, 1], f32, tag="negm")
        nc.vector.tensor_scalar_mul(negm[:], mx[:], -1.0)
        esum = smallp.tile([128, 1], f32, tag="esum")
        e_bk = smallp.tile([128, 128], f32, tag="e_bk")
        nc.scalar.activation(e_bk[:], ps_z[:], AF.Exp, bias=negm[:],
                             scale=1.0, accum_out=esum[:])
        rz = smallp.tile([128, 1], f32, tag="rz")
        nc.vector.reciprocal(rz[:], esum[:])
        u0_bk = smallp.tile([128, 128], f32, tag="u0_bk")
        nc.vector.tensor_scalar_mul(u0_bk[:], e_bk[:], rz[:])
        ps_t0 = ps0.tile([128, 128], f32, tag="ps_z")
        nc.tensor.transpose(ps_t0[:], u0_bk[:], ident_f32[:])
        nc.vector.tensor_copy(u_ring[0][:], ps_t0[:])


def _stage1_hT(nc, tc, bass, mybir, AF, Wc1s, uTall_sb, h_locs, h_alls,
               sfx=""):
    """h-col shard: this core computes h-tiles j=8c..8c+8 for ALL examples.
    Two h_loc halves [128, 4, BSZ] bf16; each AllGather issues as soon as
    its half is written so the gather overlaps the rest of stage 1/2."""
    f32 = mybir.dt.float32
    bf16 = mybir.dt.bfloat16
    with (
        tc.tile_pool(name="w1stage" + sfx, bufs=2) as w1p,
        tc.tile_pool(name="h1" + sfx, bufs=3) as h1p,
        tc.tile_pool(name="psH" + sfx, bufs=2,
                     space=bass.MemorySpace.PSUM) as psH,
    ):
        for j in range(8):
            w1t = w1p.tile([128, I1_TILES, 128], bf16, tag="w1t")
            nc.sync.dma_start(w1t[:], Wc1s[j])
            for half in range(2):
                sl = slice(half * 512, (half + 1) * 512)
                ps_h = psH.tile([128, 512], f32, tag="ps_h")
                for i in range(I1_TILES):
                    nc.tensor.matmul(ps_h[:], w1t[:, i, :],
                                     uTall_sb[:, i, sl],
                                     start=(i == 0), stop=(i == I1_TILES - 1))
                hb = h1p.tile([128, 512], bf16, tag="hb")
                nc.scalar.activation(hb[:], ps_h[:], AF.Relu)
                nc.sync.dma_start(h_locs[j // 4][:, j % 4, sl], hb[:])
            if j % 4 == 3:
                g = j // 4
                nc.gpsimd.collective_compute(
                    "AllGather", mybir.AluOpType.bypass,
                    replica_groups=[list(range(N_CORES))],
                    ins=[h_locs[g][:]], outs=[h_alls[g][:]],
                )


def _stage2_cond(nc, tc, bass, mybir, Wc2, h_alls, cond_locs,
                 cond_recvs, sfx=""):
    """cond^T col-slice for ALL examples: out[t][cc 128][ex 1024] -> DRAM.

    h_all: [8, 64, 128, B] (chunk c = examples 128c..); lhsT = W2 tile,
    moving = hT tiles [128, 512 ex]."""
    f32 = mybir.dt.float32
    bf16 = mybir.dt.bfloat16
    with (
        tc.tile_pool(name="hTh" + sfx, bufs=1) as hthp,
        tc.tile_pool(name="w2stage" + sfx, bufs=2) as w2p,
        tc.tile_pool(name="cevac" + sfx, bufs=3) as cevp,
        tc.tile_pool(name="psC" + sfx, bufs=2,
                     space=bass.MemorySpace.PSUM) as psC,
    ):
        hTh = hthp.tile([128, I2_TILES, BSZ], bf16, tag="hTh")
        # h_alls[g][s, part, j4, ex]: global h-tile i = 8 s + 4 g + j4
        for s in range(8):
            nc.sync.dma_start(hTh[:, 8 * s:8 * s + 4, :], h_alls[0][s])
            nc.sync.dma_start(hTh[:, 8 * s + 4:8 * s + 8, :], h_alls[1][s])
        nc.vector.memset(hTh[:, 64, :], 0.0)
        nc.vector.memset(hTh[0:1, 64, :], 1.0)
        # contract bias tile + AG#1-covered tiles first so matmuls start
        # while AG#2 / its gather DMAs are still in flight
        i_order = ([64]
                   + [8 * s + j for j in range(4) for s in range(8)]
                   + [8 * s + 4 + j for j in range(4) for s in range(8)])
        for t in range(NT):
            w2t = w2p.tile([128, I2_TILES, 128], bf16, tag="w2t")
            nc.sync.dma_start(w2t[:], Wc2[t])
            for half in range(2):
                ps = psC.tile([128, 512], f32, tag="ps_c")
                sl = slice(half * 512, (half + 1) * 512)
                for idx, i in enumerate(i_order):
                    nc.tensor.matmul(ps[:], w2t[:, i, :], hTh[:, i, sl],
                                     start=(idx == 0),
                                     stop=(idx == I2_TILES - 1))
                cbf = cevp.tile([128, 512], bf16, tag="cbf")
                nc.vector.tensor_copy(cbf[:], ps[:])
                # cond_locs[g] [8 exblk, 8 t, 128 cc, 128 ex]
                for eb in range(4):
                    e = half * 4 + eb
                    nc.sync.dma_start(cond_locs[t // 8][e, t % 8],
                                      cbf[:, eb * 128:(eb + 1) * 128])
            if t == 7:
                nc.gpsimd.collective_compute(
                    "AllToAll", mybir.AluOpType.bypass,
                    replica_groups=[list(range(N_CORES))],
                    ins=[cond_locs[0][:]], outs=[cond_recvs[0][:]],
                )
        nc.gpsimd.collective_compute(
            "AllToAll", mybir.AluOpType.bypass,
            replica_groups=[list(range(N_CORES))],
            ins=[cond_locs[1][:]], outs=[cond_recvs[1][:]],
        )


def _stage3_P(nc, tc, bass, mybir, AF, cfT_sb, ctT_sb, tsc_sb, P_sb, Z_sb,
              invZ_sb, sfx=""):
    """P_sb[k, b, j] = fp8(exp(trans - rowmax)); Z from the QUANTIZED values."""
    f32 = mybir.dt.float32
    with (
        tc.tile_pool(name="ptmp" + sfx, bufs=1) as ptmpp,
        tc.tile_pool(name="psB" + sfx, bufs=3,
                     space=bass.MemorySpace.PSUM) as psB,
    ):
        trb = ptmpp.tile([K, B, K], f32, tag="trb")
        for b in range(B):
            ps_tr = psB.tile([K, K], f32, tag="ps_b")
            nc.tensor.matmul(ps_tr[:], cfT_sb[:, :, :, b],
                             ctT_sb[:, :, :, b], start=True, stop=True)
            nc.vector.tensor_add(trb[:, b, :], ps_tr[:], tsc_sb[:])
        # batched softmax pieces across all examples
        mxb = ptmpp.tile([K, B], f32, tag="mxb")
        nc.vector.tensor_reduce(mxb[:], trb[:], axis=mybir.AxisListType.X,
                                op=mybir.AluOpType.max)
        nc.vector.tensor_sub(trb[:], trb[:],
                             mxb[:, :, None].broadcast_to([K, B, K]))
        nc.scalar.activation(P_sb[:], trb[:], AF.Exp)
        nc.vector.tensor_reduce(Z_sb[:], P_sb[:], axis=mybir.AxisListType.X,
                                op=mybir.AluOpType.add)
        nc.vector.reciprocal(invZ_sb[:], Z_sb[:])


def _recurrence(nc, tc, bass, mybir, AF, P_sb, Z_sb, invZ_sb, obsT, u_ring,
                Cacc, ones_col, ones_row, lenlp_sb, out, sfx=""):
    f32 = mybir.dt.float32
    bf16 = mybir.dt.bfloat16
    nc.vector.tensor_mul(u_ring[0][:], u_ring[0][:], invZ_sb[:])
    with (
        tc.tile_pool(name="obs" + sfx, bufs=6) as obsp,
        tc.tile_pool(name="rec" + sfx, bufs=2) as recp,
        tc.tile_pool(name="psR" + sfx, bufs=2,
                     space=bass.MemorySpace.PSUM) as psR,
        tc.tile_pool(name="psV" + sfx, bufs=2,
                     space=bass.MemorySpace.PSUM) as psV,
        tc.tile_pool(name="psW" + sfx, bufs=2,
                     space=bass.MemorySpace.PSUM) as psW,
    ):
        for t in range(T):
            obs_t = obsp.tile([128, L, B], bf16, tag="obs_t")
            nc.sync.dma_start(obs_t[:], obsT[t])
            e_t = recp.tile([128, L, B], f32, tag="e_t")
            nc.scalar.activation(e_t[:], obs_t[:], AF.Exp, bias=lenlp_sb[:],
                                 scale=1.0)

            lags = [l for l in range(L) if l <= t]
            pr0 = recp.tile([128, B], f32, tag="pr0")
            pr1 = recp.tile([128, B], f32, tag="pr1")
            pr2 = recp.tile([128, B], f32, tag="pr2")
            acc = None
            scratch = pr2
            for l in lags[1:][::-1]:
                pr = pr0 if acc is None else pr1
                nc.vector.tensor_mul(pr[:], u_ring[(t - l) % 7][:],
                                     e_t[:, l, :])
                if acc is None:
                    acc = pr
                else:
                    nc.vector.tensor_add(scratch[:], acc[:], pr[:])
                    acc, scratch = scratch, acc

            renorm = (t % RENORM_EVERY == RENORM_EVERY - 1)
            if not (renorm or t == T - 1):
                # lag-0 product + matvecs per half-batch: the vector work on
                # half B overlaps the tensor-engine matvecs of half A
                ps_as = psR.tile([128, B], f32, tag="ps_as")
                u_new = u_ring[(t + 1) % 7]
                for hf in range(2):
                    sl = slice(64 * hf, 64 * (hf + 1))
                    aH = recp.tile([128, 64], bf16, tag=f"a_h{hf}")
                    m0H = recp.tile([128, 64], f32, tag=f"m0_h{hf}")
                    nc.vector.tensor_mul(m0H[:], u_ring[t % 7][:, sl],
                                         e_t[:, 0, sl])
                    if acc is None:
                        nc.vector.tensor_copy(aH[:], m0H[:])
                    else:
                        nc.vector.tensor_add(aH[:], acc[:, sl], m0H[:])
                    for b in range(64 * hf, 64 * (hf + 1)):
                        nc.tensor.matmul(ps_as[:, b:b + 1], P_sb[:, b, :],
                                         aH[:, b - 64 * hf:b - 64 * hf + 1],
                                         start=True, stop=True)
                    nc.vector.tensor_mul(u_new[:, sl], ps_as[:, sl],
                                         invZ_sb[:, sl])
                continue

            a_t = recp.tile([128, B], bf16, tag="a_t")
            m0 = pr1
            nc.vector.tensor_mul(m0[:], u_ring[t % 7][:], e_t[:, 0, :])
            if acc is None:
                nc.vector.tensor_copy(a_t[:], m0[:])
            else:
                nc.vector.tensor_add(a_t[:], acc[:], m0[:])

            if t == T - 1:
                a_fix = recp.tile([128, B], bf16, tag="a_fix")
                nc.vector.tensor_mul(a_fix[:], a_t[:], Z_sb[:])
                ps_s = psV.tile([1, B], f32, tag="ps_s")
                nc.tensor.matmul(ps_s[:], ones_col[:], a_fix[:],
                                 start=True, stop=True)
                lg = recp.tile([1, B], f32, tag="lg")
                nc.scalar.activation(lg[:], ps_s[:], AF.Ln)
                outv = recp.tile([1, B], f32, tag="outv")
                nc.vector.tensor_add(outv[:], lg[:], Cacc[:])
                nc.sync.dma_start(out[:], outv[:])
                break

            if True:
                # renorm step: issue the colsum matmul BEFORE the matvec
                # block so ln/reciprocal/broadcast overlap the 128 matvecs
                ps_s = psV.tile([1, B], f32, tag="ps_s")
                nc.tensor.matmul(ps_s[:], ones_col[:], a_t[:],
                                 start=True, stop=True)
                lg = recp.tile([1, B], f32, tag="lg")
                nc.scalar.activation(lg[:], ps_s[:], AF.Ln)
                nc.vector.tensor_add(Cacc[:], Cacc[:], lg[:])
                rinv = recp.tile([1, B], f32, tag="rinv")
                nc.vector.reciprocal(rinv[:], ps_s[:])
                ps_bc = psW.tile([128, B], f32, tag="ps_bc")
                nc.tensor.matmul(ps_bc[:], ones_row[:], rinv[:],
                                 start=True, stop=True)

            ps_as = psR.tile([128, B], f32, tag="ps_as")
            for b in range(B):
                nc.tensor.matmul(ps_as[:, b:b + 1], P_sb[:, b, :],
                                 a_t[:, b:b + 1], start=True, stop=True)

            u_new = u_ring[(t + 1) % 7]
            bc = recp.tile([128, B], f32, tag="bc")
            nc.vector.tensor_copy(bc[:], ps_bc[:])
            bc2 = recp.tile([128, B], f32, tag="bc2")
            nc.vector.tensor_mul(bc2[:], bc[:], invZ_sb[:])
            nc.vector.tensor_mul(u_new[:], ps_as[:], bc2[:])
            for dj in range(1, 6):
                uj = u_ring[(t - dj + 1) % 7]
                nc.vector.tensor_mul(uj[:], uj[:], bc[:])


def _build(mode="full", reps=1):
    import concourse.bass as bass
    import concourse.tile as tile
    from concourse import mybir, masks

    f32 = mybir.dt.float32
    bf16 = mybir.dt.bfloat16
    fp8 = mybir.dt.float8e4
    AF = mybir.ActivationFunctionType

    nc = bass.Bass()
    uT = nc.dram_tensor("uT", [128, I1_TILES, B], bf16, kind="ExternalInput")
    Wi = nc.dram_tensor("Wi", [128, I1_TILES, K], bf16, kind="ExternalInput")
    Wc1s = nc.dram_tensor("Wc1s", [8, 128, I1_TILES, 128], bf16,
                          kind="ExternalInput")
    uTall = nc.dram_tensor("uTall", [128, I1_TILES, BSZ], bf16,
                           kind="ExternalInput")
    Wc2 = nc.dram_tensor("Wc2", [NT, 128, I2_TILES, 128], bf16,
                         kind="ExternalInput")
    tsc = nc.dram_tensor("tsc", [K, K], f32, kind="ExternalInput")
    obsT = nc.dram_tensor("obsT", [T, 128, L, B], bf16, kind="ExternalInput")
    out = nc.dram_tensor("out", [1, B], f32, kind="ExternalOutput")

    with tile.TileContext(nc) as tc:
        with (
            tc.tile_pool(name="const", bufs=1) as constp,
            tc.tile_pool(name="uw", bufs=1) as uwp,
            tc.tile_pool(name="small", bufs=2) as smallp,
            tc.tile_pool(name="uring", bufs=1) as uringp,
        ):
            ident_f32 = constp.tile([128, 128], f32, tag="ident_f32")
            masks.make_identity(nc, ident_f32[:])
            ones_col = constp.tile([128, 1], bf16, tag="ones_col")
            nc.vector.memset(ones_col[:], 1.0)
            ones_row = constp.tile([1, 128], f32, tag="ones_row")
            nc.vector.memset(ones_row[:], 1.0)
            tsc_sb = constp.tile([K, K], f32, tag="tsc_sb")
            nc.sync.dma_start(tsc_sb[:], tsc[:])
            Cacc = constp.tile([1, B], f32, tag="Cacc")
            lenlp_sb = constp.tile([128, 1], f32, tag="lenlp_sb")
            nc.vector.memset(lenlp_sb[:], LEN_LP)

            u_ring = [uringp.tile([K, B], f32, name=f"u{j}", tag=f"u{j}")
                      for j in range(7)]

            uT_sb = uwp.tile([128, I1_TILES, B], bf16, tag="uT_sb")
            nc.sync.dma_start(uT_sb[:], uT[:])
            uTall_sb = uwp.tile([128, I1_TILES, BSZ], bf16, tag="uTall_sb")
            nc.sync.dma_start(uTall_sb[:], uTall[:])
            Wi_sb = uwp.tile([128, I1_TILES, K], bf16, tag="Wi_sb")
            nc.sync.dma_start(Wi_sb[:], Wi[:])

            for rep in range(reps):
                sfx = f"r{rep}" if reps > 1 else ""
                h_locs = [nc.dram_tensor(f"h_loc{g}{sfx}", [128, 4, BSZ],
                                         bf16, kind="Internal")
                          for g in range(2)]
                h_alls = [nc.dram_tensor(f"h_all{g}{sfx}", [8, 128, 4, BSZ],
                                         bf16, kind="Internal",
                                         addr_space="Shared")
                          for g in range(2)]
                cond_locs = [nc.dram_tensor(f"cond_loc{g}{sfx}",
                                            [8, 8, 128, 128], bf16,
                                            kind="Internal")
                             for g in range(2)]
                cond_recvs = [nc.dram_tensor(f"cond_recv{g}{sfx}",
                                             [8, 8, 128, 128], bf16,
                                             kind="Internal")
                              for g in range(2)]

                for j in range(1, 7):
                    nc.vector.memset(u_ring[j][:], 0.0)
                nc.vector.memset(Cacc[:], 0.0)
                _stage0_pi(nc, tc, bass, mybir, AF, uT_sb, Wi_sb, smallp,
                           ident_f32, u_ring, sfx)
                _stage1_hT(nc, tc, bass, mybir, AF, Wc1s, uTall_sb,
                           h_locs, h_alls, sfx)
                _stage2_cond(nc, tc, bass, mybir, Wc2, h_alls, cond_locs,
                             cond_recvs, sfx)

                with tc.tile_pool(name="condT" + sfx, bufs=1) as condp:
                    # [a2, s, tl, ex]; global k = 16 s + tl
                    cfT_sb = condp.tile([A_DIM, 8, NT, B], bf16,
                                        tag="cfT_sb")
                    ctT_sb = condp.tile([A_DIM, 8, NT, B], bf16,
                                        tag="ctT_sb")
                    for g in range(2):
                        tl = slice(8 * g, 8 * (g + 1))
                        for src in range(8):
                            nc.sync.dma_start(
                                cfT_sb[:, src, tl, :],
                                cond_recvs[g][src, :, 0:A_DIM, :]
                                .transpose((1, 0, 2)))
                            nc.sync.dma_start(
                                ctT_sb[:, src, tl, :],
                                cond_recvs[g][src, :, A_DIM:128, :]
                                .transpose((1, 0, 2)))

                    with tc.tile_pool(name="P" + sfx, bufs=1) as Pp:
                        P_sb = Pp.tile([K, B, K], fp8, tag="P_sb")
                        Z_sb = Pp.tile([K, B], f32, tag="Z_sb")
                        invZ_sb = Pp.tile([K, B], f32, tag="invZ_sb")
                        _stage3_P(nc, tc, bass, mybir, AF, cfT_sb, ctT_sb,
                                  tsc_sb, P_sb, Z_sb, invZ_sb, sfx)
                        if mode == "phaseA":
                            with tc.tile_pool(name="fin" + sfx, bufs=1) as fp_:
                                outv = fp_.tile([1, B], f32, tag="outv")
                                nc.vector.tensor_copy(outv[:], invZ_sb[0:1, :])
                                nc.sync.dma_start(out[:], outv[:])
                        else:
                            _recurrence(nc, tc, bass, mybir, AF, P_sb, Z_sb,
                                        invZ_sb, obsT, u_ring, Cacc,
                                        ones_col, ones_row, lenlp_sb, out,
                                        sfx)

    _split_excess_waits(nc, max_waits=1)
    return nc


def _prep_inputs(uniqenc, obs_lps, W_init, b_init, A_from, A_to, W_c1, b_c1,
                 W_c2, b_c2):
    uniqenc = np.asarray(uniqenc, np.float32)
    obs_lps = np.asarray(obs_lps, np.float32)

    def fold_bias(W, b):
        Wp = np.zeros((W.shape[0] + 128, W.shape[1]), np.float32)
        Wp[:W.shape[0]] = W
        Wp[W.shape[0]] = b
        return Wp

    Wi_p = fold_bias(np.asarray(W_init, np.float32),
                     np.asarray(b_init, np.float32))          # [1152, 128]
    Wc1_p = fold_bias(np.asarray(W_c1, np.float32),
                      np.asarray(b_c1, np.float32))           # [1152, 8192]
    Wc2_p = fold_bias(np.asarray(W_c2, np.float32),
                      np.asarray(b_c2, np.float32))           # [8320, 16384]

    Wi_t = np.ascontiguousarray(
        Wi_p.reshape(I1_TILES, 128, K).transpose(1, 0, 2)).astype(BF16)
    Wc1_t = np.ascontiguousarray(
        Wc1_p.reshape(I1_TILES, 128, 64, 128).transpose(2, 1, 0, 3)).astype(BF16)
    uTall_np = np.zeros((128, I1_TILES, BSZ), np.float32)
    uTall_np[:, :8, :] = uniqenc.T.reshape(8, 128, BSZ).transpose(1, 0, 2)
    uTall_np[0, 8, :] = 1.0
    uTall_np = uTall_np.astype(BF16)
    # per-core Wc2 slice: [t 16][i2 65][128 h][128 cc]
    Wc2_r = Wc2_p.reshape(I2_TILES, 128, 128, 128)  # [i, p, tg, m]

    tsc = (np.asarray(A_from, np.float32) @ np.asarray(A_to, np.float32))
    np.fill_diagonal(tsc, NEGINF)
    tsc = np.ascontiguousarray(tsc, dtype=np.float32)

    in_maps = []
    for c in range(N_CORES):
        sl = slice(c * B, (c + 1) * B)
        uT_c = np.zeros((128, I1_TILES, B), np.float32)
        ub = uniqenc[sl].T.reshape(8, 128, B)                 # [g, p, b]
        uT_c[:, :8, :] = ub.transpose(1, 0, 2)
        uT_c[0, 8, :] = 1.0                                   # bias row 1024
        obs_c = np.zeros((T, 128, L, B), np.float32)
        for l in range(L):
            obs_c[l:, :, l, :] = obs_lps[l, :T - l, sl, :].transpose(0, 2, 1)
        Wc2_c = np.ascontiguousarray(
            Wc2_r[:, :, 16 * c:16 * (c + 1), :].transpose(2, 1, 0, 3)
        ).astype(BF16)                                        # [16, 128, 65, 128]
        in_maps.append({
            "uT": uT_c.astype(BF16),
            "Wi": Wi_t,
            "Wc1s": np.ascontiguousarray(Wc1_t[8 * c:8 * (c + 1)]),
            "uTall": uTall_np,
            "Wc2": Wc2_c,
            "tsc": tsc,
            "obsT": obs_c.astype(BF16),
        })
    return in_maps


def _get_nc():
    if "nc" not in _CACHE:
        _CACHE["nc"] = _build()
    return _CACHE["nc"]


def kernel(uniqenc, obs_lps, W_init, b_init, A_from, A_to, W_c1, b_c1,
           W_c2, b_c2):
    from concourse.bass_utils import run_bass_kernel_spmd

    nc = _get_nc()
    in_maps = _prep_inputs(uniqenc, obs_lps, W_init, b_init, A_from, A_to,
                           W_c1, b_c1, W_c2, b_c2)
    res = run_bass_kernel_spmd(nc, in_maps, core_ids=list(range(N_CORES)))
    outs = [np.asarray(res.results[c]["out"], np.float32).reshape(B)
            for c in range(N_CORES)]
    return np.concatenate(outs, axis=0)


# revision 17
# speedup vs baseline: 1.0198x; 1.0198x over previous
"""HSMM generative forward — Bass kernel v2 for 8 TRN2 NeuronCores.

v2 reshards the cond MLP with collectives to kill the 8x-replicated W_c2
HBM streaming that bounded v1:

  stage0  pi = softmax(u @ W_init)            per-core own 128 examples
  stage1  hT = relu(W_c1^T u^T)               own examples -> DRAM
  AG      AllGather h                          -> h for all 1024 examples
  stage2  cond^T slice = W_c2[:, slice]^T @ h  ALL examples, 1/8 of cols
          (per core: 16 cond-col tiles == transition-row index k)
  A2A     AllToAll cond                        -> own 128 examples, all cols
  stage3  per example: trans = tsc + cf ct^T; P = fp8(exp(z - rowmax)),
          Z = rowsum(fp8 P) [quantize-then-normalize], invZ = 1/Z
  phase B recurrence in prob space with invZ folded into the u-ring
          (astar' = invZ * P_unorm^T a), fp8 stationary weights.

W_c2 HBM traffic per core: 34 MB (was 272 MB). P matvec weight loads are
fp8 (FWL streams 4 cols/cycle vs 2 for bf16).
"""

import numpy as np
import ml_dtypes

BF16 = ml_dtypes.bfloat16

K = 128
A_DIM = 64
L = 6
T = 60
BSZ = 1024
TH2 = 1024
B = 128           # batch shard per core
N_CORES = 8
NEGINF = -1e38
LEN_LP = float(-np.log(L))

I1_TILES = 9      # ceil((1024+1)/128): uniqenc + bias row
I2_TILES = 65     # ceil((8192+1)/128): h + bias row
NT = 16           # cond-col tiles per core (== local k-tiles)
RENORM_EVERY = 6

_CACHE = {}
_WSPLIT_UID = [0]


def _split_excess_waits(nc, max_waits=1):
    """Walrus in this container rejects >1 sync wait per instruction; hoist
    excess waits onto preceding same-engine NoOps."""
    from concourse import mybir

    n_split = 0
    for fn in nc.m.functions:
        for blk in fn.blocks:
            insts = blk.instructions
            new_insts = []
            changed = False
            for inst in insts:
                si = getattr(inst, "sync_info", None)
                waits = list(si.on_wait) if si is not None else []
                if len(waits) > max_waits:
                    changed = True
                    n_split += 1
                    extra = waits[:-max_waits]
                    keep = waits[len(extra):]
                    for i in range(0, len(extra), max_waits):
                        chunk = extra[i:i + max_waits]
                        _WSPLIT_UID[0] += 1
                        nop = mybir.InstNoOp(
                            name=f"I-wsplit-{_WSPLIT_UID[0]}", ins=[], outs=[])
                        nop.engine = inst.engine
                        nop.sync_info = mybir.SyncInfo(on_wait=chunk,
                                                       on_update=[])
                        new_insts.append(nop)
                    si.on_wait = keep
                new_insts.append(inst)
            if changed:
                blk.instructions = new_insts
    return n_split


def _stage0_pi(nc, tc, bass, mybir, AF, uT_sb, Wi_sb, smallp, ident_f32,
               u_ring, sfx=""):
    f32 = mybir.dt.float32
    with tc.tile_pool(name="ps0" + sfx, bufs=2,
                      space=bass.MemorySpace.PSUM) as ps0:
        ps_z = ps0.tile([128, 128], f32, tag="ps_z")
        for i in range(I1_TILES):
            nc.tensor.matmul(ps_z[:], uT_sb[:, i, :], Wi_sb[:, i, :],
                             start=(i == 0), stop=(i == I1_TILES - 1))
        mx = smallp.tile([128, 1], f32, tag="mx")
        nc.vector.reduce_max(out=mx[:], in_=ps_z[:], axis=mybir.AxisListType.X)
        negm = smallp.tile([128, 1], f32, tag="negm")
        nc.vector.tensor_scalar_mul(negm[:], mx[:], -1.0)
        esum = smallp.tile([128, 1], f32, tag="esum")
        e_bk = smallp.tile([128, 128], f32, tag="e_bk")
        nc.scalar.activation(e_bk[:], ps_z[:], AF.Exp, bias=negm[:],
                             scale=1.0, accum_out=esum[:])
        rz = smallp.tile([128, 1], f32, tag="rz")
        nc.vector.reciprocal(rz[:], esum[:])
        u0_bk = smallp.tile([128, 128], f32, tag="u0_bk")
        nc.vector.tensor_scalar_mul(u0_bk[:], e_bk[:], rz[:])
        ps_t0 = ps0.tile([128, 128], f32, tag="ps_z")
        nc.tensor.transpose(ps_t0[:], u0_bk[:], ident_f32[:])
        nc.vector.tensor_copy(u_ring[0][:], ps_t0[:])


def _stage1_hT(nc, tc, bass, mybir, AF, Wc1s, uTall_sb, h_locs, h_alls,
               sfx=""):
    """h-col shard: this core computes h-tiles j=8c..8c+8 for ALL examples.
    Two h_loc halves [128, 4, BSZ] bf16; each AllGather issues as soon as
    its half is written so the gather overlaps the rest of stage 1/2."""
    f32 = mybir.dt.float32
    bf16 = mybir.dt.bfloat16
    with (
        tc.tile_pool(name="w1stage" + sfx, bufs=2) as w1p,
        tc.tile_pool(name="h1" + sfx, bufs=3) as h1p,
        tc.tile_pool(name="psH" + sfx, bufs=2,
                     space=bass.MemorySpace.PSUM) as psH,
    ):
        for j in range(8):
            w1t = w1p.tile([128, I1_TILES, 128], bf16, tag="w1t")
            nc.sync.dma_start(w1t[:], Wc1s[j])
            for half in range(2):
                sl = slice(half * 512, (half + 1) * 512)
                ps_h = psH.tile([128, 512], f32, tag="ps_h")
                for i in range(I1_TILES):
                    nc.tensor.matmul(ps_h[:], w1t[:, i, :],
                                     uTall_sb[:, i, sl],
                                     start=(i == 0), stop=(i == I1_TILES - 1))
                hb = h1p.tile([128, 512], bf16, tag="hb")
                nc.scalar.activation(hb[:], ps_h[:], AF.Relu)
                nc.sync.dma_start(h_locs[j // 4][:, j % 4, sl], hb[:])
            if j % 4 == 3:
                g = j // 4
                nc.gpsimd.collective_compute(
                    "AllGather", mybir.AluOpType.bypass,
                    replica_groups=[list(range(N_CORES))],
                    ins=[h_locs[g][:]], outs=[h_alls[g][:]],
                )


def _stage2_cond(nc, tc, bass, mybir, Wc2, h_alls, cond_locs,
                 cond_recvs, sfx=""):
    """cond^T col-slice for ALL examples: out[t][cc 128][ex 1024] -> DRAM.

    h_all: [8, 64, 128, B] (chunk c = examples 128c..); lhsT = W2 tile,
    moving = hT tiles [128, 512 ex]."""
    f32 = mybir.dt.float32
    bf16 = mybir.dt.bfloat16
    with (
        tc.tile_pool(name="hTh" + sfx, bufs=1) as hthp,
        tc.tile_pool(name="w2stage" + sfx, bufs=2) as w2p,
        tc.tile_pool(name="cevac" + sfx, bufs=3) as cevp,
        tc.tile_pool(name="psC" + sfx, bufs=4,
                     space=bass.MemorySpace.PSUM) as psC,
    ):
        hTh = hthp.tile([128, I2_TILES, BSZ], bf16, tag="hTh")
        # h_alls[g][s, part, j4, ex]: global h-tile i = 8 s + 4 g + j4
        for s in range(8):
            nc.sync.dma_start(hTh[:, 8 * s:8 * s + 4, :], h_alls[0][s])
            nc.sync.dma_start(hTh[:, 8 * s + 4:8 * s + 8, :], h_alls[1][s])
        nc.vector.memset(hTh[:, 64, :], 0.0)
        nc.vector.memset(hTh[0:1, 64, :], 1.0)
        # contract bias tile + AG#1-covered tiles first so matmuls start
        # while AG#2 / its gather DMAs are still in flight
        i_order = ([64]
                   + [8 * s + j for j in range(4) for s in range(8)]
                   + [8 * s + 4 + j for j in range(4) for s in range(8)])
        for t in range(NT):
            w2t = w2p.tile([128, I2_TILES, 128], bf16, tag="w2t")
            nc.sync.dma_start(w2t[:], Wc2[t])
            for half in range(2):
                ps = psC.tile([128, 512], f32, tag="ps_c")
                sl = slice(half * 512, (half + 1) * 512)
                for idx, i in enumerate(i_order):
                    nc.tensor.matmul(ps[:], w2t[:, i, :], hTh[:, i, sl],
                                     start=(idx == 0),
                                     stop=(idx == I2_TILES - 1))
                cbf = cevp.tile([128, 512], bf16, tag="cbf")
                nc.vector.tensor_copy(cbf[:], ps[:])
                # cond_locs[g] [8 exblk, tg, 128 cc, 128 ex]; g spans
                # t 0-7 / 8-11 / 12-15 so later A2As expose less latency
                g, tg = (0, t) if t < 8 else (1 + (t - 8) // 4, (t - 8) % 4)
                for eb in range(4):
                    e = half * 4 + eb
                    nc.sync.dma_start(cond_locs[g][e, tg],
                                      cbf[:, eb * 128:(eb + 1) * 128])
            if t in (7, 11):
                g = 0 if t == 7 else 1
                nc.gpsimd.collective_compute(
                    "AllToAll", mybir.AluOpType.bypass,
                    replica_groups=[list(range(N_CORES))],
                    ins=[cond_locs[g][:]], outs=[cond_recvs[g][:]],
                )
        nc.gpsimd.collective_compute(
            "AllToAll", mybir.AluOpType.bypass,
            replica_groups=[list(range(N_CORES))],
            ins=[cond_locs[2][:]], outs=[cond_recvs[2][:]],
        )


def _stage3_P(nc, tc, bass, mybir, AF, cfT_sb, ctT_sb, tsc_sb, P_sb, Z_sb,
              invZ_sb, sfx=""):
    """P_sb[k, b, j] = fp8(exp(trans - rowmax)); Z from the QUANTIZED values."""
    f32 = mybir.dt.float32
    with (
        tc.tile_pool(name="ptmp" + sfx, bufs=1) as ptmpp,
        tc.tile_pool(name="psB" + sfx, bufs=3,
                     space=bass.MemorySpace.PSUM) as psB,
    ):
        trb = ptmpp.tile([K, B, K], f32, tag="trb")
        mxb = ptmpp.tile([K, B], f32, tag="mxb")
        for hf in range(2):
            # batched softmax pieces per example-half so the vector/scalar
            # chain for half 0 overlaps the bilinears of half 1
            bs = slice(64 * hf, 64 * (hf + 1))
            for b in range(64 * hf, 64 * (hf + 1)):
                ps_tr = psB.tile([K, K], f32, tag="ps_b")
                nc.tensor.matmul(ps_tr[:], cfT_sb[:, :, :, b],
                                 ctT_sb[:, :, :, b], start=True, stop=True)
                nc.vector.tensor_add(trb[:, b, :], ps_tr[:], tsc_sb[:])
            nc.vector.tensor_reduce(mxb[:, bs], trb[:, bs, :],
                                    axis=mybir.AxisListType.X,
                                    op=mybir.AluOpType.max)
            nc.vector.tensor_sub(trb[:, bs, :], trb[:, bs, :],
                                 mxb[:, bs, None].broadcast_to([K, 64, K]))
            nc.scalar.activation(P_sb[:, bs, :], trb[:, bs, :], AF.Exp)
            nc.vector.tensor_reduce(Z_sb[:, bs], P_sb[:, bs, :],
                                    axis=mybir.AxisListType.X,
                                    op=mybir.AluOpType.add)
        nc.vector.reciprocal(invZ_sb[:], Z_sb[:])


def _recurrence(nc, tc, bass, mybir, AF, P_sb, Z_sb, invZ_sb, obsT, u_ring,
                Cacc, ones_col, ones_row, lenlp_sb, out, sfx=""):
    f32 = mybir.dt.float32
    bf16 = mybir.dt.bfloat16
    nc.vector.tensor_mul(u_ring[0][:], u_ring[0][:], invZ_sb[:])
    with (
        tc.tile_pool(name="obs" + sfx, bufs=6) as obsp,
        tc.tile_pool(name="rec" + sfx, bufs=2) as recp,
        tc.tile_pool(name="psR" + sfx, bufs=2,
                     space=bass.MemorySpace.PSUM) as psR,
        tc.tile_pool(name="psV" + sfx, bufs=2,
                     space=bass.MemorySpace.PSUM) as psV,
        tc.tile_pool(name="psW" + sfx, bufs=2,
                     space=bass.MemorySpace.PSUM) as psW,
    ):
        for t in range(T):
            obs_t = obsp.tile([128, L, B], bf16, tag="obs_t")
            nc.sync.dma_start(obs_t[:], obsT[t])
            e_t = recp.tile([128, L, B], f32, tag="e_t")
            nc.scalar.activation(e_t[:], obs_t[:], AF.Exp, bias=lenlp_sb[:],
                                 scale=1.0)

            lags = [l for l in range(L) if l <= t]
            pr0 = recp.tile([128, B], f32, tag="pr0")
            pr1 = recp.tile([128, B], f32, tag="pr1")
            pr2 = recp.tile([128, B], f32, tag="pr2")
            acc = None
            scratch = pr2
            for l in lags[1:][::-1]:
                pr = pr0 if acc is None else pr1
                nc.vector.tensor_mul(pr[:], u_ring[(t - l) % 7][:],
                                     e_t[:, l, :])
                if acc is None:
                    acc = pr
                else:
                    nc.vector.tensor_add(scratch[:], acc[:], pr[:])
                    acc, scratch = scratch, acc

            renorm = (t % RENORM_EVERY == RENORM_EVERY - 1)
            if not (renorm or t == T - 1):
                # lag-0 product + matvecs per half-batch: the vector work on
                # half B overlaps the tensor-engine matvecs of half A
                ps_as = psR.tile([128, B], f32, tag="ps_as")
                u_new = u_ring[(t + 1) % 7]
                for hf in range(2):
                    sl = slice(64 * hf, 64 * (hf + 1))
                    aH = recp.tile([128, 64], bf16, tag=f"a_h{hf}")
                    m0H = recp.tile([128, 64], f32, tag=f"m0_h{hf}")
                    nc.vector.tensor_mul(m0H[:], u_ring[t % 7][:, sl],
                                         e_t[:, 0, sl])
                    if acc is None:
                        nc.vector.tensor_copy(aH[:], m0H[:])
                    else:
                        nc.vector.tensor_add(aH[:], acc[:, sl], m0H[:])
                    for b in range(64 * hf, 64 * (hf + 1)):
                        nc.tensor.matmul(ps_as[:, b:b + 1], P_sb[:, b, :],
                                         aH[:, b - 64 * hf:b - 64 * hf + 1],
                                         start=True, stop=True)
                    nc.vector.tensor_mul(u_new[:, sl], ps_as[:, sl],
                                         invZ_sb[:, sl])
                continue

            a_t = recp.tile([128, B], bf16, tag="a_t")
            m0 = pr1
            nc.vector.tensor_mul(m0[:], u_ring[t % 7][:], e_t[:, 0, :])
            if acc is None:
                nc.vector.tensor_copy(a_t[:], m0[:])
            else:
                nc.vector.tensor_add(a_t[:], acc[:], m0[:])

            if t == T - 1:
                a_fix = recp.tile([128, B], bf16, tag="a_fix")
                nc.vector.tensor_mul(a_fix[:], a_t[:], Z_sb[:])
                ps_s = psV.tile([1, B], f32, tag="ps_s")
                nc.tensor.matmul(ps_s[:], ones_col[:], a_fix[:],
                                 start=True, stop=True)
                lg = recp.tile([1, B], f32, tag="lg")
                nc.scalar.activation(lg[:], ps_s[:], AF.Ln)
                outv = recp.tile([1, B], f32, tag="outv")
                nc.vector.tensor_add(outv[:], lg[:], Cacc[:])
                nc.sync.dma_start(out[:], outv[:])
                break

            if True:
                # renorm step: issue the colsum matmul BEFORE the matvec
                # block so ln/reciprocal/broadcast overlap the 128 matvecs
                ps_s = psV.tile([1, B], f32, tag="ps_s")
                nc.tensor.matmul(ps_s[:], ones_col[:], a_t[:],
                                 start=True, stop=True)
                lg = recp.tile([1, B], f32, tag="lg")
                nc.scalar.activation(lg[:], ps_s[:], AF.Ln)
                nc.vector.tensor_add(Cacc[:], Cacc[:], lg[:])
                rinv = recp.tile([1, B], f32, tag="rinv")
                nc.vector.reciprocal(rinv[:], ps_s[:])
                ps_bc = psW.tile([128, B], f32, tag="ps_bc")
                nc.tensor.matmul(ps_bc[:], ones_row[:], rinv[:],
                                 start=True, stop=True)

            ps_as = psR.tile([128, B], f32, tag="ps_as")
            for b in range(B):
                nc.tensor.matmul(ps_as[:, b:b + 1], P_sb[:, b, :],
                                 a_t[:, b:b + 1], start=True, stop=True)

            u_new = u_ring[(t + 1) % 7]
            bc = recp.tile([128, B], f32, tag="bc")
            nc.vector.tensor_copy(bc[:], ps_bc[:])
            bc2 = recp.tile([128, B], f32, tag="bc2")
            nc.vector.tensor_mul(bc2[:], bc[:], invZ_sb[:])
            nc.vector.tensor_mul(u_new[:], ps_as[:], bc2[:])
            for dj in range(1, 6):
                uj = u_ring[(t - dj + 1) % 7]
                nc.vector.tensor_mul(uj[:], uj[:], bc[:])


def _build(mode="full", reps=1):
    import concourse.bass as bass
    import concourse.tile as tile
    from concourse import mybir, masks

    f32 = mybir.dt.float32
    bf16 = mybir.dt.bfloat16
    fp8 = mybir.dt.float8e4
    AF = mybir.ActivationFunctionType

    nc = bass.Bass()
    uT = nc.dram_tensor("uT", [128, I1_TILES, B], bf16, kind="ExternalInput")
    Wi = nc.dram_tensor("Wi", [128, I1_TILES, K], bf16, kind="ExternalInput")
    Wc1s = nc.dram_tensor("Wc1s", [8, 128, I1_TILES, 128], bf16,
                          kind="ExternalInput")
    uTall = nc.dram_tensor("uTall", [128, I1_TILES, BSZ], bf16,
                           kind="ExternalInput")
    Wc2 = nc.dram_tensor("Wc2", [NT, 128, I2_TILES, 128], bf16,
                         kind="ExternalInput")
    tsc = nc.dram_tensor("tsc", [K, K], f32, kind="ExternalInput")
    obsT = nc.dram_tensor("obsT", [T, 128, L, B], bf16, kind="ExternalInput")
    out = nc.dram_tensor("out", [1, B], f32, kind="ExternalOutput")

    with tile.TileContext(nc) as tc:
        with (
            tc.tile_pool(name="const", bufs=1) as constp,
            tc.tile_pool(name="uw", bufs=1) as uwp,
            tc.tile_pool(name="small", bufs=2) as smallp,
            tc.tile_pool(name="uring", bufs=1) as uringp,
        ):
            ident_f32 = constp.tile([128, 128], f32, tag="ident_f32")
            masks.make_identity(nc, ident_f32[:])
            ones_col = constp.tile([128, 1], bf16, tag="ones_col")
            nc.vector.memset(ones_col[:], 1.0)
            ones_row = constp.tile([1, 128], f32, tag="ones_row")
            nc.vector.memset(ones_row[:], 1.0)
            tsc_sb = constp.tile([K, K], f32, tag="tsc_sb")
            nc.sync.dma_start(tsc_sb[:], tsc[:])
            Cacc = constp.tile([1, B], f32, tag="Cacc")
            lenlp_sb = constp.tile([128, 1], f32, tag="lenlp_sb")
            nc.vector.memset(lenlp_sb[:], LEN_LP)

            u_ring = [uringp.tile([K, B], f32, name=f"u{j}", tag=f"u{j}")
                      for j in range(7)]

            uT_sb = uwp.tile([128, I1_TILES, B], bf16, tag="uT_sb")
            nc.sync.dma_start(uT_sb[:], uT[:])
            uTall_sb = uwp.tile([128, I1_TILES, BSZ], bf16, tag="uTall_sb")
            for i in range(I1_TILES):
                nc.sync.dma_start(uTall_sb[:, i, :], uTall[:, i, :])
            Wi_sb = uwp.tile([128, I1_TILES, K], bf16, tag="Wi_sb")
            nc.sync.dma_start(Wi_sb[:], Wi[:])

            for rep in range(reps):
                sfx = f"r{rep}" if reps > 1 else ""
                h_locs = [nc.dram_tensor(f"h_loc{g}{sfx}", [128, 4, BSZ],
                                         bf16, kind="Internal")
                          for g in range(2)]
                h_alls = [nc.dram_tensor(f"h_all{g}{sfx}", [8, 128, 4, BSZ],
                                         bf16, kind="Internal",
                                         addr_space="Shared")
                          for g in range(2)]
                _ntg = [8, 4, 4]
                cond_locs = [nc.dram_tensor(f"cond_loc{g}{sfx}",
                                            [8, _ntg[g], 128, 128], bf16,
                                            kind="Internal")
                             for g in range(3)]
                cond_recvs = [nc.dram_tensor(f"cond_recv{g}{sfx}",
                                             [8, _ntg[g], 128, 128], bf16,
                                             kind="Internal")
                              for g in range(3)]

                for j in range(1, 7):
                    nc.vector.memset(u_ring[j][:], 0.0)
                nc.vector.memset(Cacc[:], 0.0)
                _stage0_pi(nc, tc, bass, mybir, AF, uT_sb, Wi_sb, smallp,
                           ident_f32, u_ring, sfx)
                _stage1_hT(nc, tc, bass, mybir, AF, Wc1s, uTall_sb,
                           h_locs, h_alls, sfx)
                _stage2_cond(nc, tc, bass, mybir, Wc2, h_alls, cond_locs,
                             cond_recvs, sfx)

                with tc.tile_pool(name="condT" + sfx, bufs=1) as condp:
                    # [a2, s, tl, ex]; global k = 16 s + tl
                    cfT_sb = condp.tile([A_DIM, 8, NT, B], bf16,
                                        tag="cfT_sb")
                    ctT_sb = condp.tile([A_DIM, 8, NT, B], bf16,
                                        tag="ctT_sb")
                    _tl0 = [0, 8, 12]
                    for g in range(3):
                        tl = slice(_tl0[g], _tl0[g] + (8 if g == 0 else 4))
                        for src in range(8):
                            nc.sync.dma_start(
                                cfT_sb[:, src, tl, :],
                                cond_recvs[g][src, :, 0:A_DIM, :]
                                .transpose((1, 0, 2)))
                            nc.sync.dma_start(
                                ctT_sb[:, src, tl, :],
                                cond_recvs[g][src, :, A_DIM:128, :]
                                .transpose((1, 0, 2)))

                    with tc.tile_pool(name="P" + sfx, bufs=1) as Pp:
                        P_sb = Pp.tile([K, B, K], fp8, tag="P_sb")
                        Z_sb = Pp.tile([K, B], f32, tag="Z_sb")
                        invZ_sb = Pp.tile([K, B], f32, tag="invZ_sb")
                        _stage3_P(nc, tc, bass, mybir, AF, cfT_sb, ctT_sb,
                                  tsc_sb, P_sb, Z_sb, invZ_sb, sfx)
                        if mode == "phaseA":
                            with tc.tile_pool(name="fin" + sfx, bufs=1) as fp_:
                                outv = fp_.tile([1, B], f32, tag="outv")
                                nc.vector.tensor_copy(outv[:], invZ_sb[0:1, :])
                                nc.sync.dma_start(out[:], outv[:])
                        else:
                            _recurrence(nc, tc, bass, mybir, AF, P_sb, Z_sb,
                                        invZ_sb, obsT, u_ring, Cacc,
                                        ones_col, ones_row, lenlp_sb, out,
                                        sfx)

    _split_excess_waits(nc, max_waits=1)
    return nc


def _prep_inputs(uniqenc, obs_lps, W_init, b_init, A_from, A_to, W_c1, b_c1,
                 W_c2, b_c2):
    uniqenc = np.asarray(uniqenc, np.float32)
    obs_lps = np.asarray(obs_lps, np.float32)

    def fold_bias(W, b):
        Wp = np.zeros((W.shape[0] + 128, W.shape[1]), np.float32)
        Wp[:W.shape[0]] = W
        Wp[W.shape[0]] = b
        return Wp

    Wi_p = fold_bias(np.asarray(W_init, np.float32),
                     np.asarray(b_init, np.float32))          # [1152, 128]
    Wc1_p = fold_bias(np.asarray(W_c1, np.float32),
                      np.asarray(b_c1, np.float32))           # [1152, 8192]
    Wc2_p = fold_bias(np.asarray(W_c2, np.float32),
                      np.asarray(b_c2, np.float32))           # [8320, 16384]

    Wi_t = np.ascontiguousarray(
        Wi_p.reshape(I1_TILES, 128, K).transpose(1, 0, 2)).astype(BF16)
    Wc1_t = np.ascontiguousarray(
        Wc1_p.reshape(I1_TILES, 128, 64, 128).transpose(2, 1, 0, 3)).astype(BF16)
    uTall_np = np.zeros((128, I1_TILES, BSZ), np.float32)
    uTall_np[:, :8, :] = uniqenc.T.reshape(8, 128, BSZ).transpose(1, 0, 2)
    uTall_np[0, 8, :] = 1.0
    uTall_np = uTall_np.astype(BF16)
    # per-core Wc2 slice: [t 16][i2 65][128 h][128 cc]
    Wc2_r = Wc2_p.reshape(I2_TILES, 128, 128, 128)  # [i, p, tg, m]

    tsc = (np.asarray(A_from, np.float32) @ np.asarray(A_to, np.float32))
    np.fill_diagonal(tsc, NEGINF)
    tsc = np.ascontiguousarray(tsc, dtype=np.float32)

    in_maps = []
    for c in range(N_CORES):
        sl = slice(c * B, (c + 1) * B)
        uT_c = np.zeros((128, I1_TILES, B), np.float32)
        ub = uniqenc[sl].T.reshape(8, 128, B)                 # [g, p, b]
        uT_c[:, :8, :] = ub.transpose(1, 0, 2)
        uT_c[0, 8, :] = 1.0                                   # bias row 1024
        obs_c = np.zeros((T, 128, L, B), np.float32)
        for l in range(L):
            obs_c[l:, :, l, :] = obs_lps[l, :T - l, sl, :].transpose(0, 2, 1)
        Wc2_c = np.ascontiguousarray(
            Wc2_r[:, :, 16 * c:16 * (c + 1), :].transpose(2, 1, 0, 3)
        ).astype(BF16)                                        # [16, 128, 65, 128]
        in_maps.append({
            "uT": uT_c.astype(BF16),
            "Wi": Wi_t,
            "Wc1s": np.ascontiguousarray(Wc1_t[8 * c:8 * (c + 1)]),
            "uTall": uTall_np,
            "Wc2": Wc2_c,
            "tsc": tsc,
            "obsT": obs_c.astype(BF16),
        })
    return in_maps


def _get_nc():
    if "nc" not in _CACHE:
        _CACHE["nc"] = _build()
    return _CACHE["nc"]


def kernel(uniqenc, obs_lps, W_init, b_init, A_from, A_to, W_c1, b_c1,
           W_c2, b_c2):
    from concourse.bass_utils import run_bass_kernel_spmd

    nc = _get_nc()
    in_maps = _prep_inputs(uniqenc, obs_lps, W_init, b_init, A_from, A_to,
                           W_c1, b_c1, W_c2, b_c2)
    res = run_bass_kernel_spmd(nc, in_maps, core_ids=list(range(N_CORES)))
    outs = [np.asarray(res.results[c]["out"], np.float32).reshape(B)
            for c in range(N_CORES)]
    return np.concatenate(outs, axis=0)


# revision 18
# speedup vs baseline: 1.0260x; 1.0060x over previous
"""HSMM generative forward — Bass kernel v2 for 8 TRN2 NeuronCores.

v2 reshards the cond MLP with collectives to kill the 8x-replicated W_c2
HBM streaming that bounded v1:

  stage0  pi = softmax(u @ W_init)            per-core own 128 examples
  stage1  hT = relu(W_c1^T u^T)               own examples -> DRAM
  AG      AllGather h                          -> h for all 1024 examples
  stage2  cond^T slice = W_c2[:, slice]^T @ h  ALL examples, 1/8 of cols
          (per core: 16 cond-col tiles == transition-row index k)
  A2A     AllToAll cond                        -> own 128 examples, all cols
  stage3  per example: trans = tsc + cf ct^T; P = fp8(exp(z - rowmax)),
          Z = rowsum(fp8 P) [quantize-then-normalize], invZ = 1/Z
  phase B recurrence in prob space with invZ folded into the u-ring
          (astar' = invZ * P_unorm^T a), fp8 stationary weights.

W_c2 HBM traffic per core: 34 MB (was 272 MB). P matvec weight loads are
fp8 (FWL streams 4 cols/cycle vs 2 for bf16).
"""

import numpy as np
import ml_dtypes

BF16 = ml_dtypes.bfloat16

K = 128
A_DIM = 64
L = 6
T = 60
BSZ = 1024
TH2 = 1024
B = 128           # batch shard per core
N_CORES = 8
NEGINF = -1e38
LEN_LP = float(-np.log(L))

I1_TILES = 9      # ceil((1024+1)/128): uniqenc + bias row
I2_TILES = 65     # ceil((8192+1)/128): h + bias row
NT = 16           # cond-col tiles per core (== local k-tiles)
RENORM_EVERY = 6

_CACHE = {}
_WSPLIT_UID = [0]


def _split_excess_waits(nc, max_waits=1):
    """Walrus in this container rejects >1 sync wait per instruction; hoist
    excess waits onto preceding same-engine NoOps."""
    from concourse import mybir

    n_split = 0
    for fn in nc.m.functions:
        for blk in fn.blocks:
            insts = blk.instructions
            new_insts = []
            changed = False
            for inst in insts:
                si = getattr(inst, "sync_info", None)
                waits = list(si.on_wait) if si is not None else []
                if len(waits) > max_waits:
                    changed = True
                    n_split += 1
                    extra = waits[:-max_waits]
                    keep = waits[len(extra):]
                    for i in range(0, len(extra), max_waits):
                        chunk = extra[i:i + max_waits]
                        _WSPLIT_UID[0] += 1
                        nop = mybir.InstNoOp(
                            name=f"I-wsplit-{_WSPLIT_UID[0]}", ins=[], outs=[])
                        nop.engine = inst.engine
                        nop.sync_info = mybir.SyncInfo(on_wait=chunk,
                                                       on_update=[])
                        new_insts.append(nop)
                    si.on_wait = keep
                new_insts.append(inst)
            if changed:
                blk.instructions = new_insts
    return n_split


def _stage0_pi(nc, tc, bass, mybir, AF, uT_sb, Wi_sb, smallp, ident_f32,
               u_ring, sfx=""):
    f32 = mybir.dt.float32
    with tc.tile_pool(name="ps0" + sfx, bufs=2,
                      space=bass.MemorySpace.PSUM) as ps0:
        ps_z = ps0.tile([128, 128], f32, tag="ps_z")
        for i in range(I1_TILES):
            nc.tensor.matmul(ps_z[:], uT_sb[:, i, :], Wi_sb[:, i, :],
                             start=(i == 0), stop=(i == I1_TILES - 1))
        mx = smallp.tile([128, 1], f32, tag="mx")
        nc.vector.reduce_max(out=mx[:], in_=ps_z[:], axis=mybir.AxisListType.X)
        negm = smallp.tile([128, 1], f32, tag="negm")
        nc.vector.tensor_scalar_mul(negm[:], mx[:], -1.0)
        esum = smallp.tile([128, 1], f32, tag="esum")
        e_bk = smallp.tile([128, 128], f32, tag="e_bk")
        nc.scalar.activation(e_bk[:], ps_z[:], AF.Exp, bias=negm[:],
                             scale=1.0, accum_out=esum[:])
        rz = smallp.tile([128, 1], f32, tag="rz")
        nc.vector.reciprocal(rz[:], esum[:])
        u0_bk = smallp.tile([128, 128], f32, tag="u0_bk")
        nc.vector.tensor_scalar_mul(u0_bk[:], e_bk[:], rz[:])
        ps_t0 = ps0.tile([128, 128], f32, tag="ps_z")
        nc.tensor.transpose(ps_t0[:], u0_bk[:], ident_f32[:])
        nc.vector.tensor_copy(u_ring[0][:], ps_t0[:])


def _stage1_hT(nc, tc, bass, mybir, AF, Wc1s, uTall_sb, h_locs, h_alls,
               sfx=""):
    """h-col shard: this core computes h-tiles j=8c..8c+8 for ALL examples.
    Two h_loc halves [128, 4, BSZ] bf16; each AllGather issues as soon as
    its half is written so the gather overlaps the rest of stage 1/2."""
    f32 = mybir.dt.float32
    bf16 = mybir.dt.bfloat16
    with (
        tc.tile_pool(name="w1stage" + sfx, bufs=2) as w1p,
        tc.tile_pool(name="h1" + sfx, bufs=3) as h1p,
        tc.tile_pool(name="psH" + sfx, bufs=2,
                     space=bass.MemorySpace.PSUM) as psH,
    ):
        for j in range(8):
            w1t = w1p.tile([128, I1_TILES, 128], bf16, tag="w1t")
            nc.sync.dma_start(w1t[:], Wc1s[j])
            for half in range(2):
                sl = slice(half * 512, (half + 1) * 512)
                ps_h = psH.tile([128, 512], f32, tag="ps_h")
                for i in range(I1_TILES):
                    nc.tensor.matmul(ps_h[:], w1t[:, i, :],
                                     uTall_sb[:, i, sl],
                                     start=(i == 0), stop=(i == I1_TILES - 1))
                hb = h1p.tile([128, 512], bf16, tag="hb")
                nc.scalar.activation(hb[:], ps_h[:], AF.Relu)
                nc.sync.dma_start(h_locs[j // 4][:, j % 4, sl], hb[:])
            if j % 4 == 3:
                g = j // 4
                nc.gpsimd.collective_compute(
                    "AllGather", mybir.AluOpType.bypass,
                    replica_groups=[list(range(N_CORES))],
                    ins=[h_locs[g][:]], outs=[h_alls[g][:]],
                )


def _stage2_cond(nc, tc, bass, mybir, Wc2, h_alls, cond_locs,
                 cond_recvs, sfx=""):
    """cond^T col-slice for ALL examples: out[t][cc 128][ex 1024] -> DRAM.

    h_all: [8, 64, 128, B] (chunk c = examples 128c..); lhsT = W2 tile,
    moving = hT tiles [128, 512 ex]."""
    f32 = mybir.dt.float32
    bf16 = mybir.dt.bfloat16
    with (
        tc.tile_pool(name="hTh" + sfx, bufs=1) as hthp,
        tc.tile_pool(name="w2stage" + sfx, bufs=2) as w2p,
        tc.tile_pool(name="cevac" + sfx, bufs=3) as cevp,
        tc.tile_pool(name="psC" + sfx, bufs=4,
                     space=bass.MemorySpace.PSUM) as psC,
    ):
        hTh = hthp.tile([128, I2_TILES, BSZ], bf16, tag="hTh")
        # h_alls[g][s, part, j4, ex]: global h-tile i = 8 s + 4 g + j4
        for s in range(8):
            nc.sync.dma_start(hTh[:, 8 * s:8 * s + 4, :], h_alls[0][s])
            nc.sync.dma_start(hTh[:, 8 * s + 4:8 * s + 8, :], h_alls[1][s])
        nc.vector.memset(hTh[:, 64, :], 0.0)
        nc.vector.memset(hTh[0:1, 64, :], 1.0)
        # contract bias tile + AG#1-covered tiles first so matmuls start
        # while AG#2 / its gather DMAs are still in flight
        i_order = ([64]
                   + [8 * s + j for j in range(4) for s in range(8)]
                   + [8 * s + 4 + j for j in range(4) for s in range(8)])
        for t in range(NT):
            w2t = w2p.tile([128, I2_TILES, 128], bf16, tag="w2t")
            nc.sync.dma_start(w2t[:], Wc2[t])
            for half in range(2):
                ps = psC.tile([128, 512], f32, tag="ps_c")
                sl = slice(half * 512, (half + 1) * 512)
                for idx, i in enumerate(i_order):
                    nc.tensor.matmul(ps[:], w2t[:, i, :], hTh[:, i, sl],
                                     start=(idx == 0),
                                     stop=(idx == I2_TILES - 1))
                cbf = cevp.tile([128, 512], bf16, tag="cbf")
                nc.vector.tensor_copy(cbf[:], ps[:])
                # cond_locs[g] [8 exblk, tg, 128 cc, 128 ex]; g spans
                # t 0-7 / 8-11 / 12-13 / 14-15: later A2As expose less
                if t < 8:
                    g, tg = 0, t
                elif t < 12:
                    g, tg = 1, t - 8
                else:
                    g, tg = 2 + (t - 12) // 2, (t - 12) % 2
                for eb in range(4):
                    e = half * 4 + eb
                    nc.sync.dma_start(cond_locs[g][e, tg],
                                      cbf[:, eb * 128:(eb + 1) * 128])
            if t in (7, 11, 13):
                g = {7: 0, 11: 1, 13: 2}[t]
                nc.gpsimd.collective_compute(
                    "AllToAll", mybir.AluOpType.bypass,
                    replica_groups=[list(range(N_CORES))],
                    ins=[cond_locs[g][:]], outs=[cond_recvs[g][:]],
                )
        nc.gpsimd.collective_compute(
            "AllToAll", mybir.AluOpType.bypass,
            replica_groups=[list(range(N_CORES))],
            ins=[cond_locs[3][:]], outs=[cond_recvs[3][:]],
        )


def _stage3_P(nc, tc, bass, mybir, AF, cfT_sb, ctT_sb, tsc_sb, P_sb, Z_sb,
              invZ_sb, sfx=""):
    """P_sb[k, b, j] = fp8(exp(trans - rowmax)); Z from the QUANTIZED values."""
    f32 = mybir.dt.float32
    with (
        tc.tile_pool(name="ptmp" + sfx, bufs=1) as ptmpp,
        tc.tile_pool(name="psB" + sfx, bufs=3,
                     space=bass.MemorySpace.PSUM) as psB,
    ):
        trb = ptmpp.tile([K, B, K], f32, tag="trb")
        mxb = ptmpp.tile([K, B], f32, tag="mxb")
        for hf in range(2):
            # batched softmax pieces per example-half so the vector/scalar
            # chain for half 0 overlaps the bilinears of half 1
            bs = slice(64 * hf, 64 * (hf + 1))
            for b in range(64 * hf, 64 * (hf + 1)):
                ps_tr = psB.tile([K, K], f32, tag="ps_b")
                nc.tensor.matmul(ps_tr[:], cfT_sb[:, :, :, b],
                                 ctT_sb[:, :, :, b], start=True, stop=True)
                nc.vector.tensor_add(trb[:, b, :], ps_tr[:], tsc_sb[:])
            nc.vector.tensor_reduce(mxb[:, bs], trb[:, bs, :],
                                    axis=mybir.AxisListType.X,
                                    op=mybir.AluOpType.max)
            nc.vector.tensor_sub(trb[:, bs, :], trb[:, bs, :],
                                 mxb[:, bs, None].broadcast_to([K, 64, K]))
            nc.scalar.activation(P_sb[:, bs, :], trb[:, bs, :], AF.Exp)
            nc.vector.tensor_reduce(Z_sb[:, bs], P_sb[:, bs, :],
                                    axis=mybir.AxisListType.X,
                                    op=mybir.AluOpType.add)
        nc.vector.reciprocal(invZ_sb[:], Z_sb[:])


def _recurrence(nc, tc, bass, mybir, AF, P_sb, Z_sb, invZ_sb, obsT, u_ring,
                Cacc, ones_col, ones_row, lenlp_sb, out, sfx=""):
    f32 = mybir.dt.float32
    bf16 = mybir.dt.bfloat16
    nc.vector.tensor_mul(u_ring[0][:], u_ring[0][:], invZ_sb[:])
    with (
        tc.tile_pool(name="obs" + sfx, bufs=6) as obsp,
        tc.tile_pool(name="rec" + sfx, bufs=2) as recp,
        tc.tile_pool(name="psR" + sfx, bufs=4,
                     space=bass.MemorySpace.PSUM) as psR,
        tc.tile_pool(name="psV" + sfx, bufs=2,
                     space=bass.MemorySpace.PSUM) as psV,
        tc.tile_pool(name="psW" + sfx, bufs=2,
                     space=bass.MemorySpace.PSUM) as psW,
    ):
        for t in range(T):
            obs_t = obsp.tile([128, L, B], bf16, tag="obs_t")
            nc.sync.dma_start(obs_t[:], obsT[t])
            e_t = recp.tile([128, L, B], f32, tag="e_t")
            nc.scalar.activation(e_t[:], obs_t[:], AF.Exp, bias=lenlp_sb[:],
                                 scale=1.0)

            lags = [l for l in range(L) if l <= t]
            pr0 = recp.tile([128, B], f32, tag="pr0")
            pr1 = recp.tile([128, B], f32, tag="pr1")
            pr2 = recp.tile([128, B], f32, tag="pr2")
            acc = None
            scratch = pr2
            for l in lags[1:][::-1]:
                pr = pr0 if acc is None else pr1
                nc.vector.tensor_mul(pr[:], u_ring[(t - l) % 7][:],
                                     e_t[:, l, :])
                if acc is None:
                    acc = pr
                else:
                    nc.vector.tensor_add(scratch[:], acc[:], pr[:])
                    acc, scratch = scratch, acc

            renorm = (t % RENORM_EVERY == RENORM_EVERY - 1)
            if not (renorm or t == T - 1):
                # lag-0 product + matvecs per half-batch: the vector work on
                # half B overlaps the tensor-engine matvecs of half A
                ps_as = psR.tile([128, B], f32, tag="ps_as")
                u_new = u_ring[(t + 1) % 7]
                for hf in range(2):
                    sl = slice(64 * hf, 64 * (hf + 1))
                    aH = recp.tile([128, 64], bf16, tag=f"a_h{hf}")
                    m0H = recp.tile([128, 64], f32, tag=f"m0_h{hf}")
                    nc.vector.tensor_mul(m0H[:], u_ring[t % 7][:, sl],
                                         e_t[:, 0, sl])
                    if acc is None:
                        nc.vector.tensor_copy(aH[:], m0H[:])
                    else:
                        nc.vector.tensor_add(aH[:], acc[:, sl], m0H[:])
                    for b in range(64 * hf, 64 * (hf + 1)):
                        nc.tensor.matmul(ps_as[:, b:b + 1], P_sb[:, b, :],
                                         aH[:, b - 64 * hf:b - 64 * hf + 1],
                                         start=True, stop=True)
                    nc.vector.tensor_mul(u_new[:, sl], ps_as[:, sl],
                                         invZ_sb[:, sl])
                continue

            a_t = recp.tile([128, B], bf16, tag="a_t")
            m0 = pr1
            nc.vector.tensor_mul(m0[:], u_ring[t % 7][:], e_t[:, 0, :])
            if acc is None:
                nc.vector.tensor_copy(a_t[:], m0[:])
            else:
                nc.vector.tensor_add(a_t[:], acc[:], m0[:])

            if t == T - 1:
                a_fix = recp.tile([128, B], bf16, tag="a_fix")
                nc.vector.tensor_mul(a_fix[:], a_t[:], Z_sb[:])
                ps_s = psV.tile([1, B], f32, tag="ps_s")
                nc.tensor.matmul(ps_s[:], ones_col[:], a_fix[:],
                                 start=True, stop=True)
                lg = recp.tile([1, B], f32, tag="lg")
                nc.scalar.activation(lg[:], ps_s[:], AF.Ln)
                outv = recp.tile([1, B], f32, tag="outv")
                nc.vector.tensor_add(outv[:], lg[:], Cacc[:])
                nc.sync.dma_start(out[:], outv[:])
                break

            if True:
                # renorm step: issue the colsum matmul BEFORE the matvec
                # block so ln/reciprocal/broadcast overlap the 128 matvecs
                ps_s = psV.tile([1, B], f32, tag="ps_s")
                nc.tensor.matmul(ps_s[:], ones_col[:], a_t[:],
                                 start=True, stop=True)
                lg = recp.tile([1, B], f32, tag="lg")
                nc.scalar.activation(lg[:], ps_s[:], AF.Ln)
                nc.vector.tensor_add(Cacc[:], Cacc[:], lg[:])
                rinv = recp.tile([1, B], f32, tag="rinv")
                nc.vector.reciprocal(rinv[:], ps_s[:])
                ps_bc = psW.tile([128, B], f32, tag="ps_bc")
                nc.tensor.matmul(ps_bc[:], ones_row[:], rinv[:],
                                 start=True, stop=True)

            ps_as = psR.tile([128, B], f32, tag="ps_as")
            for b in range(B):
                nc.tensor.matmul(ps_as[:, b:b + 1], P_sb[:, b, :],
                                 a_t[:, b:b + 1], start=True, stop=True)

            u_new = u_ring[(t + 1) % 7]
            bc = recp.tile([128, B], f32, tag="bc")
            nc.vector.tensor_copy(bc[:], ps_bc[:])
            bc2 = recp.tile([128, B], f32, tag="bc2")
            nc.vector.tensor_mul(bc2[:], bc[:], invZ_sb[:])
            nc.vector.tensor_mul(u_new[:], ps_as[:], bc2[:])
            for dj in range(1, 6):
                uj = u_ring[(t - dj + 1) % 7]
                nc.vector.tensor_mul(uj[:], uj[:], bc[:])


def _build(mode="full", reps=1):
    import concourse.bass as bass
    import concourse.tile as tile
    from concourse import mybir, masks

    f32 = mybir.dt.float32
    bf16 = mybir.dt.bfloat16
    fp8 = mybir.dt.float8e4
    AF = mybir.ActivationFunctionType

    nc = bass.Bass()
    uT = nc.dram_tensor("uT", [128, I1_TILES, B], bf16, kind="ExternalInput")
    Wi = nc.dram_tensor("Wi", [128, I1_TILES, K], bf16, kind="ExternalInput")
    Wc1s = nc.dram_tensor("Wc1s", [8, 128, I1_TILES, 128], bf16,
                          kind="ExternalInput")
    uTall = nc.dram_tensor("uTall", [128, I1_TILES, BSZ], bf16,
                           kind="ExternalInput")
    Wc2 = nc.dram_tensor("Wc2", [NT, 128, I2_TILES, 128], bf16,
                         kind="ExternalInput")
    tsc = nc.dram_tensor("tsc", [K, K], f32, kind="ExternalInput")
    obsT = nc.dram_tensor("obsT", [T, 128, L, B], bf16, kind="ExternalInput")
    out = nc.dram_tensor("out", [1, B], f32, kind="ExternalOutput")

    with tile.TileContext(nc) as tc:
        with (
            tc.tile_pool(name="const", bufs=1) as constp,
            tc.tile_pool(name="uw", bufs=1) as uwp,
            tc.tile_pool(name="small", bufs=2) as smallp,
            tc.tile_pool(name="uring", bufs=1) as uringp,
        ):
            ident_f32 = constp.tile([128, 128], f32, tag="ident_f32")
            masks.make_identity(nc, ident_f32[:])
            ones_col = constp.tile([128, 1], bf16, tag="ones_col")
            nc.vector.memset(ones_col[:], 1.0)
            ones_row = constp.tile([1, 128], f32, tag="ones_row")
            nc.vector.memset(ones_row[:], 1.0)
            tsc_sb = constp.tile([K, K], f32, tag="tsc_sb")
            nc.sync.dma_start(tsc_sb[:], tsc[:])
            Cacc = constp.tile([1, B], f32, tag="Cacc")
            lenlp_sb = constp.tile([128, 1], f32, tag="lenlp_sb")
            nc.vector.memset(lenlp_sb[:], LEN_LP)

            u_ring = [uringp.tile([K, B], f32, name=f"u{j}", tag=f"u{j}")
                      for j in range(7)]

            uT_sb = uwp.tile([128, I1_TILES, B], bf16, tag="uT_sb")
            nc.sync.dma_start(uT_sb[:], uT[:])
            uTall_sb = uwp.tile([128, I1_TILES, BSZ], bf16, tag="uTall_sb")
            for i in range(I1_TILES):
                nc.sync.dma_start(uTall_sb[:, i, :], uTall[:, i, :])
            Wi_sb = uwp.tile([128, I1_TILES, K], bf16, tag="Wi_sb")
            nc.sync.dma_start(Wi_sb[:], Wi[:])

            for rep in range(reps):
                sfx = f"r{rep}" if reps > 1 else ""
                h_locs = [nc.dram_tensor(f"h_loc{g}{sfx}", [128, 4, BSZ],
                                         bf16, kind="Internal")
                          for g in range(2)]
                h_alls = [nc.dram_tensor(f"h_all{g}{sfx}", [8, 128, 4, BSZ],
                                         bf16, kind="Internal",
                                         addr_space="Shared")
                          for g in range(2)]
                _ntg = [8, 4, 2, 2]
                cond_locs = [nc.dram_tensor(f"cond_loc{g}{sfx}",
                                            [8, _ntg[g], 128, 128], bf16,
                                            kind="Internal")
                             for g in range(4)]
                cond_recvs = [nc.dram_tensor(f"cond_recv{g}{sfx}",
                                             [8, _ntg[g], 128, 128], bf16,
                                             kind="Internal")
                              for g in range(4)]

                for j in range(1, 7):
                    nc.vector.memset(u_ring[j][:], 0.0)
                nc.vector.memset(Cacc[:], 0.0)
                _stage0_pi(nc, tc, bass, mybir, AF, uT_sb, Wi_sb, smallp,
                           ident_f32, u_ring, sfx)
                _stage1_hT(nc, tc, bass, mybir, AF, Wc1s, uTall_sb,
                           h_locs, h_alls, sfx)
                _stage2_cond(nc, tc, bass, mybir, Wc2, h_alls, cond_locs,
                             cond_recvs, sfx)

                with tc.tile_pool(name="condT" + sfx, bufs=1) as condp:
                    # [a2, s, tl, ex]; global k = 16 s + tl
                    cfT_sb = condp.tile([A_DIM, 8, NT, B], bf16,
                                        tag="cfT_sb")
                    ctT_sb = condp.tile([A_DIM, 8, NT, B], bf16,
                                        tag="ctT_sb")
                    _tl0 = [0, 8, 12, 14]
                    _tln = [8, 4, 2, 2]
                    for g in range(4):
                        tl = slice(_tl0[g], _tl0[g] + _tln[g])
                        for src in range(8):
                            nc.sync.dma_start(
                                cfT_sb[:, src, tl, :],
                                cond_recvs[g][src, :, 0:A_DIM, :]
                                .transpose((1, 0, 2)))
                            nc.sync.dma_start(
                                ctT_sb[:, src, tl, :],
                                cond_recvs[g][src, :, A_DIM:128, :]
                                .transpose((1, 0, 2)))

                    with tc.tile_pool(name="P" + sfx, bufs=1) as Pp:
                        P_sb = Pp.tile([K, B, K], fp8, tag="P_sb")
                        Z_sb = Pp.tile([K, B], f32, tag="Z_sb")
                        invZ_sb = Pp.tile([K, B], f32, tag="invZ_sb")
                        _stage3_P(nc, tc, bass, mybir, AF, cfT_sb, ctT_sb,
                                  tsc_sb, P_sb, Z_sb, invZ_sb, sfx)
                        if mode == "phaseA":
                            with tc.tile_pool(name="fin" + sfx, bufs=1) as fp_:
                                outv = fp_.tile([1, B], f32, tag="outv")
                                nc.vector.tensor_copy(outv[:], invZ_sb[0:1, :])
                                nc.sync.dma_start(out[:], outv[:])
                        else:
                            _recurrence(nc, tc, bass, mybir, AF, P_sb, Z_sb,
                                        invZ_sb, obsT, u_ring, Cacc,
                                        ones_col, ones_row, lenlp_sb, out,
                                        sfx)

    _split_excess_waits(nc, max_waits=1)
    return nc


def _prep_inputs(uniqenc, obs_lps, W_init, b_init, A_from, A_to, W_c1, b_c1,
                 W_c2, b_c2):
    uniqenc = np.asarray(uniqenc, np.float32)
    obs_lps = np.asarray(obs_lps, np.float32)

    def fold_bias(W, b):
        Wp = np.zeros((W.shape[0] + 128, W.shape[1]), np.float32)
        Wp[:W.shape[0]] = W
        Wp[W.shape[0]] = b
        return Wp

    Wi_p = fold_bias(np.asarray(W_init, np.float32),
                     np.asarray(b_init, np.float32))          # [1152, 128]
    Wc1_p = fold_bias(np.asarray(W_c1, np.float32),
                      np.asarray(b_c1, np.float32))           # [1152, 8192]
    Wc2_p = fold_bias(np.asarray(W_c2, np.float32),
                      np.asarray(b_c2, np.float32))           # [8320, 16384]

    Wi_t = np.ascontiguousarray(
        Wi_p.reshape(I1_TILES, 128, K).transpose(1, 0, 2)).astype(BF16)
    Wc1_t = np.ascontiguousarray(
        Wc1_p.reshape(I1_TILES, 128, 64, 128).transpose(2, 1, 0, 3)).astype(BF16)
    uTall_np = np.zeros((128, I1_TILES, BSZ), np.float32)
    uTall_np[:, :8, :] = uniqenc.T.reshape(8, 128, BSZ).transpose(1, 0, 2)
    uTall_np[0, 8, :] = 1.0
    uTall_np = uTall_np.astype(BF16)
    # per-core Wc2 slice: [t 16][i2 65][128 h][128 cc]
    Wc2_r = Wc2_p.reshape(I2_TILES, 128, 128, 128)  # [i, p, tg, m]

    tsc = (np.asarray(A_from, np.float32) @ np.asarray(A_to, np.float32))
    np.fill_diagonal(tsc, NEGINF)
    tsc = np.ascontiguousarray(tsc, dtype=np.float32)

    in_maps = []
    for c in range(N_CORES):
        sl = slice(c * B, (c + 1) * B)
        uT_c = np.zeros((128, I1_TILES, B), np.float32)
        ub = uniqenc[sl].T.reshape(8, 128, B)                 # [g, p, b]
        uT_c[:, :8, :] = ub.transpose(1, 0, 2)
        uT_c[0, 8, :] = 1.0                                   # bias row 1024
        obs_c = np.zeros((T, 128, L, B), np.float32)
        for l in range(L):
            obs_c[l:, :, l, :] = obs_lps[l, :T - l, sl, :].transpose(0, 2, 1)
        Wc2_c = np.ascontiguousarray(
            Wc2_r[:, :, 16 * c:16 * (c + 1), :].transpose(2, 1, 0, 3)
        ).astype(BF16)                                        # [16, 128, 65, 128]
        in_maps.append({
            "uT": uT_c.astype(BF16),
            "Wi": Wi_t,
            "Wc1s": np.ascontiguousarray(Wc1_t[8 * c:8 * (c + 1)]),
            "uTall": uTall_np,
            "Wc2": Wc2_c,
            "tsc": tsc,
            "obsT": obs_c.astype(BF16),
        })
    return in_maps


def _get_nc():
    if "nc" not in _CACHE:
        _CACHE["nc"] = _build()
    return _CACHE["nc"]


def kernel(uniqenc, obs_lps, W_init, b_init, A_from, A_to, W_c1, b_c1,
           W_c2, b_c2):
    from concourse.bass_utils import run_bass_kernel_spmd

    nc = _get_nc()
    in_maps = _prep_inputs(uniqenc, obs_lps, W_init, b_init, A_from, A_to,
                           W_c1, b_c1, W_c2, b_c2)
    res = run_bass_kernel_spmd(nc, in_maps, core_ids=list(range(N_CORES)))
    outs = [np.asarray(res.results[c]["out"], np.float32).reshape(B)
            for c in range(N_CORES)]
    return np.concatenate(outs, axis=0)
